# revision 35
# baseline (speedup 1.0000x reference)
"""Multi-head attention on 8 Trainium2 NeuronCores.

Problem: x[2, 2048, 1024] -> qkv proj (w_qkv [1024, 3072], 16 heads x 64) ->
softmax attention -> out proj (w_out [1024, 1024] + b_out).

Sharding: core c in 0..7 handles batch b = c // 4 and heads 4*(c%4) .. 4*(c%4)+3.
Each core computes a partial output projection over its 4 heads' slice; the four
cores of each batch group ReduceScatter(add) the partials chunk-by-chunk in bf16
(bias/4 folded in on every core) directly into the bf16 output parameter,
overlapped with later attention chunks. The host reassembles + casts to f32.

Schedule (v2): the softmax exp on ScalarE (~133us busy) is the critical
engine; everything else is organized to keep it 100% fed:
  - scores are emitted as 64x128 row-tiled pairs (tile T0/T8) -> true 2x
    concurrency in the PE array (microbenched: 110ns/MM vs 216 serial)
  - PSUM: 3 rotating [128,1024] "st" slots (scores/exp units) + 2 [65,512]
    AV accumulators = 8 banks exactly; all projections borrow "st" slots
  - K-proj chunk 0 + q-proj(0,0) run in a short phase 1; K-proj c1-3 and
    the whole V-proj are interleaved into block (0,0)'s groups
  - per-block tail (last AV + PSUM evac + reciprocal chain) is deferred
    into the next block so it never blocks the PE queue at a boundary
  - out-proj tiles spread 4-per-block; final 512 rows ReduceScatter in
    4x128-row chunks so the collective tail is short
  - a few f32r warmup matmuls run during the initial DMA to flip the PE
    HAM clock gate to 2.4GHz before the projections start
"""

import numpy as np

N = 2048          # sequence length per batch
D = 1024          # model dim
DH = 64           # head dim
HPC = 4           # heads per core
NCORES = 8
GSIZE = 4         # cores per reduce group
SCALE = DH ** -0.5
NCH = N // 512    # query chunks

# ReduceScatter chunks: (row0, rows). Collectives serialize on the CC core
# (~3us overhead + ~11us/MB each), so keep few chunks.
RS_CHUNKS = [(0, 512), (512, 512), (1024, 512), (1536, 512)]
RS_OFF = [0, 128, 256, 384]  # y_out row offset per chunk

_cached = {}


def _build_nc():
    from contextlib import ExitStack

    import concourse.bacc as bacc
    import concourse.mybir as mybir
    from concourse import tile

    f32 = mybir.dt.float32
    f32r = mybir.dt.float32r
    bf16 = mybir.dt.bfloat16

    nc = bacc.Bacc(num_devices=NCORES)

    xT = nc.declare_dram_parameter("xT", [D, N], bf16, isOutput=False)
    # wkq columns: [k pair0 | k pair1 | q pair0 | q pair1] (128 each)
    wkq = nc.declare_dram_parameter("wkq", [D, 2 * HPC * DH], bf16, isOutput=False)
    wv = nc.declare_dram_parameter("wv", [D, HPC * DH], bf16, isOutput=False)
    wout = nc.declare_dram_parameter("wout", [HPC * DH, D], bf16, isOutput=False)
    bias = nc.declare_dram_parameter("bias", [1, D], f32r, isOutput=False)
    ones1 = nc.declare_dram_parameter("ones1", [1, 128], f32r, isOutput=False)
    quart = nc.declare_dram_parameter("quart", [1, 128], f32r, isOutput=False)
    y_out = nc.declare_dram_parameter("y", [512, D], bf16, isOutput=True)

    KB = D // 128           # 8 contraction blocks for the projections
    JB = N // 128           # 16 key blocks
    VW = DH + 1             # v columns per head incl. ones column

    with tile.TileContext(nc) as tc:
        ctx = ExitStack()
        with ctx:
            sb = ctx.enter_context(tc.tile_pool(name="sb", bufs=1))
            ps_big = ctx.enter_context(tc.tile_pool(name="ps_big", bufs=3, space="PSUM"))
            ps_o = ctx.enter_context(tc.tile_pool(name="ps_o", bufs=2, space="PSUM"))
            dram = ctx.enter_context(tc.tile_pool(name="dram", bufs=1, space="DRAM"))
            sb_attn = ctx.enter_context(tc.tile_pool(name="sb_attn", bufs=6))
            sb_work = ctx.enter_context(tc.tile_pool(name="sb_work", bufs=10))
            otmp_pool = ctx.enter_context(tc.tile_pool(name="otmp", bufs=4))

            # persistent SBUF residents
            k_sb = sb.tile([128, 2, N], bf16, tag="k")         # kT pair0 / pair1
            q_sb = sb.tile([128, 4, 512], bf16, tag="q")       # double-buffered qT
            v_sb = sb.tile([128, JB, HPC * VW], bf16, tag="v")
            # o2: head pair q stacked on partitions (even head 0-63, odd 64-127)
            # so the out-proj is 2 K=128 matmuls per tile (sums over heads)
            o2_sb = sb.tile([128, 2, N], bf16, tag="o")
            wo2_sb = sb.tile([128, 2, D], bf16, tag="wo")
            bias_bc = sb.tile([128, D], f32, tag="bias_bc")
            bias_sb = sb.tile([1, D], f32r, tag="bias")
            ones_sb = sb.tile([1, 128], f32r, tag="ones1")
            ones_bf = sb.tile([1, 128], bf16, tag="ones_bf")
            quart_sb = sb.tile([1, 128], f32r, tag="quart")
            xT_sb = sb.tile([128, KB, N], bf16, tag="xT")
            wkq_sb = sb.tile([128, KB, 2 * HPC * DH], bf16, tag="wkq")
            wv_sb = sb.tile([128, KB, HPC * DH], bf16, tag="wv")

            # ---- DMA priority order ----
            nc.sync.dma_start(out=ones_sb[:], in_=ones1[:, :])
            with nc.allow_low_precision(reason="bf16 ones for the recip bcast"):
                nc.vector.tensor_copy(ones_bf[:], ones_sb[:])
            nc.sync.dma_start(out=quart_sb[:], in_=quart[:, :])
            nc.sync.dma_start(out=bias_sb[:], in_=bias[:, :])
            for kb in range(KB):   # k+q0 cols + xT chunk 0 (tokens 0-511)
                nc.sync.dma_start(out=wkq_sb[:, kb, 0:256],
                                  in_=wkq[kb * 128:(kb + 1) * 128, 0:256])
                nc.sync.dma_start(out=xT_sb[:, kb, 0:512],
                                  in_=xT[kb * 128:(kb + 1) * 128, 0:512])
                nc.sync.dma_start(out=wkq_sb[:, kb, 256:384],
                                  in_=wkq[kb * 128:(kb + 1) * 128, 256:384])
            for kb in range(KB):   # wv (vproj starts in block (0,0) g0)
                nc.sync.dma_start(out=wv_sb[:, kb, :],
                                  in_=wv[kb * 128:(kb + 1) * 128, :])
            for kb in range(KB):   # xT chunk 1 (kproj c1 at g0)
                nc.sync.dma_start(out=xT_sb[:, kb, 512:1024],
                                  in_=xT[kb * 128:(kb + 1) * 128, 512:1024])
            for kb in range(KB):   # q pair1 cols
                nc.sync.dma_start(out=wkq_sb[:, kb, 384:512],
                                  in_=wkq[kb * 128:(kb + 1) * 128, 384:512])
            for c in range(2, 4):
                for kb in range(KB):
                    nc.sync.dma_start(out=xT_sb[:, kb, c * 512:(c + 1) * 512],
                                      in_=xT[kb * 128:(kb + 1) * 128, c * 512:(c + 1) * 512])
            for q in range(2):
                nc.sync.dma_start(out=wo2_sb[:, q, :],
                                  in_=wout[q * 128:(q + 1) * 128, :])

            nc.vector.memset(v_sb[:], 1.0)  # preset ones columns

            # ---- PE warmup during the initial DMA (flip HAM to 2.4GHz) ----
            warm_ps = ps_big.tile([128, 1024], f32, tag="st", name="warm")
            for i in range(8):
                nc.tensor.matmul(warm_ps[:, 0:512], ones_sb[:], bias_sb[:, 0:512],
                                 start=(i == 0), stop=(i == 7))
            # dummy activation so the exp table set loads during phase 1
            warm_act = sb_work.tile([128, 16], f32, tag="warmact", name="warmact")
            nc.scalar.activation(warm_act[:], warm_ps[:, 0:16],
                                 mybir.ActivationFunctionType.Exp, scale=1.0)
            warm_out = sb_work.tile([128, 16], bf16, tag="warmout", name="warmout")
            with nc.allow_low_precision(reason="warmup junk"):
                nc.vector.tensor_copy(warm_out[:], warm_act[:])
            warm_sink = dram.tile([128, 16], bf16, tag="warm_sink", name="warm_sink")
            nc.sync.dma_start(out=warm_sink[:], in_=warm_out[:])

            # ---- bias/4 broadcast to 128 partitions (PE K=1 matmul) ----
            for ch in range(2):
                bps = ps_big.tile([128, 1024], f32, tag="st", name=f"bps{ch}")
                nc.tensor.matmul(bps[:, 0:512], quart_sb[:],
                                 bias_sb[:, ch * 512:(ch + 1) * 512],
                                 start=True, stop=True)
                nc.vector.tensor_copy(bias_bc[:, ch * 512:(ch + 1) * 512], bps[:, 0:512])

            def emit_kproj(c):
                """kT for key chunk c (512 keys) for both head pairs."""
                for mb in range(2):
                    mps = ps_big.tile([128, 1024], f32, tag="st", name=f"kps{mb}_{c}")
                    for kb in range(KB):
                        nc.tensor.matmul(
                            mps[:, 0:512],
                            wkq_sb[:, kb, mb * 128:(mb + 1) * 128],
                            xT_sb[:, kb, c * 512:(c + 1) * 512],
                            start=(kb == 0), stop=(kb == KB - 1))
                    nc.vector.tensor_copy(k_sb[:, mb, c * 512:(c + 1) * 512], mps[:, 0:512])

            def qslot(ich, pair):
                return 2 * (ich % 2) + pair

            _qpend = {}

            def emit_qproj_half(ich, pair, half):
                """qT for query chunk ich, half 0: alloc + 4 MMs; half 1: rest."""
                mb = 2 + pair
                if half == 0:
                    _qpend[(ich, pair)] = ps_big.tile(
                        [128, 1024], f32, tag="st", name=f"qps{ich}_{pair}")
                mps = _qpend[(ich, pair)]
                for kb in range(half * 4, half * 4 + 4):
                    nc.tensor.matmul(
                        mps[:, 0:512],
                        wkq_sb[:, kb, mb * 128:(mb + 1) * 128],
                        xT_sb[:, kb, ich * 512:(ich + 1) * 512],
                        start=(kb == 0), stop=(kb == KB - 1))
                if half == 1:
                    nc.vector.tensor_copy(q_sb[:, qslot(ich, pair), :], mps[:, 0:512])
                    del _qpend[(ich, pair)]

            def emit_qproj(ich, pair):
                emit_qproj_half(ich, pair, 0)
                emit_qproj_half(ich, pair, 1)

            def emit_vproj(jb):
                """v rows for key block jb, all 4 heads (+ preserved ones cols)."""
                vps = ps_big.tile([128, 1024], f32, tag="st", name=f"vps{jb}")
                for kb in range(KB):
                    nc.tensor.matmul(
                        vps[:, 0:256],
                        xT_sb[:, kb, jb * 128:(jb + 1) * 128],
                        wv_sb[:, kb, :],
                        start=(kb == 0), stop=(kb == KB - 1))
                nc.vector.tensor_copy(
                    v_sb[:, jb, :].rearrange("p (h c) -> p h c", c=VW)[:, :, 0:DH],
                    vps[:, 0:256].rearrange("p (h c) -> p h c", c=DH))

            # ---- output projection tiles + ReduceScatter plumbing ----
            y_parts = [dram.tile([r, D], bf16, tag=f"y_part{c}", name=f"y_part{c}")
                       for c, (_, r) in enumerate(RS_CHUNKS)]
            y_reds = [dram.tile([r // 4, D], bf16, tag=f"y_red{c}", name=f"y_red{c}")
                      for c, (_, r) in enumerate(RS_CHUNKS)]
            groups = [[0, 1, 2, 3], [4, 5, 6, 7]]

            def _chunk_of(ib):
                row = ib * 128
                for c, (r0, r) in enumerate(RS_CHUNKS):
                    if r0 <= row < r0 + r:
                        return c, row - r0
                raise AssertionError(ib)

            def emit_proj_tile(ib, ec):
                """one 128x512 tile of the output projection (+ bias/4, bf16)"""
                ibs = slice(ib * 128, (ib + 1) * 128)
                yps = ps_big.tile([128, 1024], f32, tag="st", name=f"yps{ib}_{ec}")
                for q in range(2):
                    nc.tensor.matmul(
                        yps[:, 0:512],
                        o2_sb[:, q, ibs],
                        wo2_sb[:, q, ec * 512:(ec + 1) * 512],
                        start=(q == 0), stop=(q == 1))
                ysb = sb_work.tile([128, 512], bf16, tag="y", name=f"ysb{ib}_{ec}")
                with nc.allow_low_precision(reason="bf16 partials for the reduce-scatter"):
                    nc.vector.tensor_add(ysb[:], yps[:, 0:512],
                                         bias_bc[:, ec * 512:(ec + 1) * 512])
                c, roff = _chunk_of(ib)
                nc.sync.dma_start(
                    out=y_parts[c][roff:roff + 128, ec * 512:(ec + 1) * 512],
                    in_=ysb[:])

            def emit_rs(c):
                shard = RS_CHUNKS[c][1] // 4
                nc.gpsimd.collective_compute(
                    "ReduceScatter",
                    mybir.AluOpType.add,
                    replica_groups=groups,
                    ins=[y_parts[c][:]],
                    outs=[y_reds[c][:]],
                )
                # collectives may not write IO tensors; bounce DRAM->DRAM.
                # On the GpSimd queue: it waits for the RS, and the Sync queue
                # (proj-tile DMAs) must not block behind that wait.
                nc.gpsimd.dma_start(out=y_out[RS_OFF[c]:RS_OFF[c] + shard, :],
                                    in_=y_reds[c][:])

            # ---- attention blocks ----
            state = {}    # (ich, pair) -> {h: (rden, otmp)}
            avstate = {}  # (ich, pair) -> {"ops": .., "ats": ..}

            def emit_norm_finish(ich, pair, s):
                """broadcast 1/denom over 128 partitions and scale one head"""
                isl = slice(ich * 512, (ich + 1) * 512)
                h = pair * 2 + s
                rden, ot = state[(ich, pair)].pop(h)
                if not state[(ich, pair)]:
                    del state[(ich, pair)]
                rps = ps_big.tile([128, 1024], f32, tag="st", name=f"rps{h}_{ich}")
                nc.tensor.matmul(rps[:, 0:512], ones_bf[:], rden[:],
                                 start=True, stop=True)
                # even head -> partitions 0-63 of plane `pair`; odd head needs
                # partitions 64-127, which DVE lanes can't shift to -- bounce
                # through an SBUF->SBUF DMA partition remap
                with nc.allow_low_precision(reason="bf16 attention output"):
                    if s == 0:
                        nc.vector.tensor_mul(o2_sb[0:64, pair, isl],
                                             ot[0:64, :], rps[0:64, 0:512])
                    else:
                        osh = sb_work.tile([64, 512], bf16, tag="osh",
                                           name=f"osh{h}_{ich}")
                        nc.vector.tensor_mul(osh[:], ot[0:64, :], rps[0:64, 0:512])
                        nc.sync.dma_start(out=o2_sb[64:128, pair, isl], in_=osh[:])

            def emit_av(ich, pair, g):
                """AV matmuls consuming exp'd scores of group g (jb 2g, 2g+1)."""
                st = avstate[(ich, pair)]
                for s in range(2):
                    h = pair * 2 + s
                    for u in range(2):
                        jb = 2 * g + u
                        nc.tensor.matmul(
                            st["ops"][s][:],
                            v_sb[:, jb, h * VW:(h + 1) * VW],
                            st["ats"][s, g][:, u * 512:(u + 1) * 512],
                            start=(jb == 0), stop=(jb == JB - 1))

            def emit_block_tail(ich, pair):
                """last AV group + PSUM evac + reciprocal chain for a block."""
                st = avstate.pop((ich, pair))
                st8 = {}
                ots = {}
                for s in range(2):
                    h = pair * 2 + s
                    ot = otmp_pool.tile([65, 512], f32, tag="otmp", name=f"otmp{h}_{ich}")
                    nc.vector.tensor_copy(ot[:], st["ops"][s][:])
                    ots[s] = ot
                # pack the two 512-wide denom rows across 128 partitions via
                # SBUF->SBUF DMA so the reciprocal runs on all lanes, then
                # scatter back as f32r rows
                # pack/scatter DMAs ride the Sync queue: on the Scalar queue
                # they would stall later exps, and the GpSimd queue is blocked
                # for the full duration of each collective
                dpack = sb_work.tile([128, 8], f32, tag="dpack", name=f"dpack{ich}_{pair}")
                for s in range(2):
                    nc.sync.dma_start(out=dpack[:, s * 4:(s + 1) * 4],
                                      in_=ots[s][64:65, :])
                rpack = sb_work.tile([128, 8], f32, tag="rpack", name=f"rpack{ich}_{pair}")
                nc.vector.reciprocal(rpack[:], dpack[:])
                rpb = sb_work.tile([128, 8], bf16, tag="rpb", name=f"rpb{ich}_{pair}")
                with nc.allow_low_precision(reason="bf16 softmax denom recip"):
                    nc.vector.tensor_copy(rpb[:], rpack[:])
                for s in range(2):
                    h = pair * 2 + s
                    rden = sb_work.tile([1, 512], bf16, tag="rden", name=f"rden{h}_{ich}")
                    nc.sync.dma_start(out=rden[:], in_=rpb[:, s * 4:(s + 1) * 4])
                    st8[h] = (rden, ots[s])
                state[(ich, pair)] = st8

            def emit_scores(ich, pair, g):
                """row-tiled paired score MMs + exp for group g -> at tiles."""
                st = avstate[(ich, pair)]
                qsl = qslot(ich, pair)
                sts = {}
                for s in range(2):
                    sts[s] = ps_big.tile([128, 1024], f32, tag="st",
                                         name=f"st{pair * 2 + s}_{ich}_{g}")
                for u in range(2):
                    jb = 2 * g + u
                    for s in range(2):
                        psl = slice(s * 64, s * 64 + 64)
                        nc.tensor.matmul(
                            sts[s][:, u * 512:(u + 1) * 512],
                            k_sb[psl, pair, jb * 128:(jb + 1) * 128],
                            q_sb[psl, qsl, :],
                            start=True, stop=True)
                for s in range(2):
                    h = pair * 2 + s
                    at = sb_attn.tile([128, 1024], bf16, tag="attn",
                                      name=f"at{h}_{ich}_{g}")
                    nc.scalar.activation(at[:], sts[s][:],
                                         mybir.ActivationFunctionType.Exp,
                                         scale=float(SCALE))
                    st["ats"][s, g] = at

            def begin_block(ich, pair):
                ops = {}
                for s in range(2):
                    h = pair * 2 + s
                    ops[s] = ps_o.tile([65, 512], f32, tag="o", name=f"ops{h}_{ich}")
                avstate[(ich, pair)] = {"ops": ops, "ats": {}}

            # ---- phase 1: kproj chunk 0 + qproj(0,0) ----
            emit_kproj(0)
            emit_qproj(0, 0)

            # ---- main loop ----
            # per-block insert schedule: at most ONE extra "st" PSUM allocation
            # per group so the scores/exp slot rotation keeps its lookahead.
            blocks = [(ich, pair) for ich in range(NCH) for pair in range(2)]
            for idx, (ich, pair) in enumerate(blocks):
                prev = blocks[idx - 1] if idx >= 1 else None
                begin_block(ich, pair)
                # proj tiles of chunk ich-1 handled by this block.
                # pair 0 hosts 3 at g5-g7 (they need this block's g2/g3 norms);
                # pair 1 hosts 5 at g0/g1/g5/g6/g7 (its norms requirement was
                # met by the g2/g3 norms of the pair-0 block). For ich=3 the
                # split is 6/2 so RS(2) can issue early in (3,1) and the CC
                # core is free before the drain collective.
                projs0, projs1 = [], []
                if ich >= 1:
                    tiles = [((ich - 1) * 4 + t // 2, t % 2) for t in range(8)]
                    if ich == 3:
                        projs0, projs1 = tiles[:6], tiles[6:]
                    else:
                        projs0, projs1 = tiles[:3], tiles[3:]
                qp = None  # which qproj this block emits (as one g4 burst)
                if pair == 0 and ich >= 1:
                    qp = (ich, 1)
                elif pair == 1 and ich + 1 < NCH:
                    qp = (ich + 1, 0)
                if idx == 1:
                    qp = (1, 0)
                # group -> insert list; at most one st-alloc per group
                ins = {g: [] for g in range(8)}
                if prev is not None:
                    ins[2].append(lambda p=prev: emit_norm_finish(p[0], p[1], 0))
                    ins[3].append(lambda p=prev: emit_norm_finish(p[0], p[1], 1))
                if qp is not None and idx >= 1:
                    ins[4].append(lambda a=qp: emit_qproj(a[0], a[1]))
                projs = projs0 if pair == 0 else projs1
                if pair == 0:
                    pslots = [5, 6, 7, 6, 7, 5]
                else:
                    pslots = [0, 1, 5, 6, 7]
                for i, (ib, ec) in enumerate(projs):
                    ins[pslots[i]].append(lambda a=ib, b=ec: emit_proj_tile(a, b))
                if pair == 1 and ich in (1, 2):
                    ins[7].append(lambda c=ich - 1: emit_rs(c))
                if ich == 3 and pair == 1:
                    ins[1].append(lambda: emit_rs(2))
                for g in range(8):
                    emit_scores(ich, pair, g)
                    if g == 0 and prev is not None:
                        emit_av(prev[0], prev[1], 7)   # deferred tail AV
                        emit_block_tail(*prev)
                    if g >= 1:
                        emit_av(ich, pair, g - 1)
                    if idx == 0:
                        emit_vproj(2 * g)
                        emit_vproj(2 * g + 1)
                        if g <= 2:
                            emit_kproj(g + 1)
                        if g == 3:
                            emit_qproj(0, 1)
                    else:
                        for fn in ins[g]:
                            fn()

            # ---- drain ----
            last = blocks[-1]
            emit_av(last[0], last[1], 7)
            emit_block_tail(*last)
            emit_norm_finish(last[0], last[1], 0)
            emit_norm_finish(last[0], last[1], 1)
            for ib in (12, 13, 14, 15):
                emit_proj_tile(ib, 0)
                emit_proj_tile(ib, 1)
            emit_rs(3)

    nc.finalize()
    return nc


def _make_in_maps(x, w_qkv, w_out, b_out):
    import ml_dtypes

    bf = ml_dtypes.bfloat16
    x = np.asarray(x, dtype=np.float32)
    w_qkv = np.asarray(w_qkv, dtype=np.float32)
    w_out = np.asarray(w_out, dtype=np.float32)
    b_out = np.asarray(b_out, dtype=np.float32)
    ones1 = np.ones((1, 128), dtype=np.float32)
    quart = np.full((1, 128), 0.25, dtype=np.float32)
    in_maps = []
    for c in range(NCORES):
        b = c // GSIZE
        h0 = (c % GSIZE) * HPC
        cols = np.arange(h0 * DH, (h0 + HPC) * DH)
        wq = w_qkv[:, cols]
        wk = w_qkv[:, D + cols]
        wv = w_qkv[:, 2 * D + cols]
        in_maps.append({
            "xT": np.ascontiguousarray(x[b].T).astype(bf),
            "wkq": np.ascontiguousarray(np.concatenate([wk, wq], axis=1)).astype(bf),
            "wv": np.ascontiguousarray(wv).astype(bf),
            "wout": np.ascontiguousarray(w_out[cols, :]).astype(bf),
            "bias": b_out[None, :],
            "ones1": ones1,
            "quart": quart,
        })
    return in_maps


def _assemble(results, x_shape):
    B = x_shape[0]
    y = np.empty((B, N, D), dtype=np.float32)
    for b in range(B):
        for g in range(GSIZE):
            shard = np.asarray(results[b * GSIZE + g]["y"]).astype(np.float32)
            for c, (r0, r) in enumerate(RS_CHUNKS):
                sz = r // 4
                y[b, r0 + g * sz: r0 + (g + 1) * sz, :] = \
                    shard[RS_OFF[c]:RS_OFF[c] + sz, :]
    return y


def kernel(x, w_qkv, w_out, b_out):
    from concourse.bass_utils import run_bass_kernel_spmd

    if "nc" not in _cached:
        _cached["nc"] = _build_nc()
    nc = _cached["nc"]
    in_maps = _make_in_maps(x, w_qkv, w_out, b_out)
    res = run_bass_kernel_spmd(nc, in_maps, list(range(NCORES)))
    return _assemble(res.results, np.asarray(x).shape)
